# revision 30
# baseline (speedup 1.0000x reference)
"""GAT (3-layer, PPI-style) Bass/Tile kernel for 8 Trainium2 NeuronCores.

Strategy (graph/data parallel, dst-ownership sharding):
  - Nodes sharded contiguously: core c owns nodes [c*NOWN, (c+1)*NOWN).
  - Per layer: Phase A computes [feat | el | er] for owned nodes with one
    fp16 matmul against W_aug = [W | W@al_bd | W@ar_bd]; an fp16 AllGather
    publishes [feat, el] rows (768B / 1024B rows) to every core; er for
    owned nodes is spread into a local 256B-row table (er_wide).
  - Edge phase: edges grouped by 128-node dst groups, split into two
    streams by src table half (int16 gather indices), padded per
    (group, stream) to 128-slot tiles with tile counts = max over cores
    (one SPMD program serves all cores). Window-batched SWDGE dma_gather
    calls (multi-group, single_packet=False, round-robin over 4 SWDGE
    queues for ~4x gather bandwidth) fetch feat+el by src and er by dst.
    Per dst group, ONE batched is_equal builds the one-hot matrices for
    all tiles; softmax numerator and denominator come from one matmul
    per tile (denominator rides as extra rhs columns); all element-wise
    work (logits, exp, alpha*feat) is batched across the group's tiles.
  - out[n] = ps[:, :FT] * recip(ps[:, FT:]) with ELU + fp16 PE transpose
    producing the next layer's x^T.

All graph-dependent index structures are computed on the host inside
kernel() and shipped as tensor inputs; per-group tile counts are baked
into the compiled program (recompiled if the graph changes).
"""

import math

import numpy as np

P = 128
NCORES = 8
NQ = 4  # SWDGE queues
WIN = 8  # tiles per gather window (per stream)
FBUFS = 4  # windows in flight per stream
ERW = 128  # er_wide row elements (fp16) -> 256B rows


# ----------------------------------------------------------------------------
# Host-side preparation
# ----------------------------------------------------------------------------


def _wrap_idxs(idx):
    """int16 index array for dma_gather: [n*16] -> [128, n] (wrapped in 16
    partitions, replicated 8x)."""
    n = idx.shape[0] // 16
    w = idx.astype(np.int16).reshape(n, 16).T  # [16, n]
    return np.tile(w, (8, 1))


def _prepare(inputs):
    h = np.asarray(inputs["h"], dtype=np.float32)
    src = np.asarray(inputs["src"]).astype(np.int64)
    dst = np.asarray(inputs["dst"]).astype(np.int64)

    N, NFEAT = h.shape
    E = src.shape[0]
    assert N % NCORES == 0
    NOWN = N // NCORES
    G = math.ceil(NOWN / P)
    # table chunks: chunk0 = first C0G groups of each core (stream A),
    # chunk1 = the rest (stream B); AllGather runs per chunk.
    C0G = (G + 1) // 2
    C0N = min(C0G * P, NOWN)
    C1N = NOWN - C0N
    assert NCORES * max(C0N, C1N) <= 32767 and NOWN <= 32767

    Ws, als, ars = [], [], []
    for i in (1, 2, 3):
        Ws.append(np.asarray(inputs[f"W{i}"], dtype=np.float32))
        als.append(np.asarray(inputs[f"al{i}"], dtype=np.float32))
        ars.append(np.asarray(inputs[f"ar{i}"], dtype=np.float32))
    H = als[0].shape[0]
    FEAT = [W.shape[1] for W in Ws]
    D = [f // H for f in FEAT]
    NCLASS = D[-1]

    # W_aug = [W | W @ al_bd | W @ ar_bd]
    Waug = []
    for W, al, ar, f, d in zip(Ws, als, ars, FEAT, D):
        al_bd = np.zeros((f, H), dtype=np.float32)
        ar_bd = np.zeros((f, H), dtype=np.float32)
        for hh in range(H):
            al_bd[hh * d : (hh + 1) * d, hh] = al[hh]
            ar_bd[hh * d : (hh + 1) * d, hh] = ar[hh]
        Waug.append(
            np.ascontiguousarray(
                np.concatenate([W, W @ al_bd, W @ ar_bd], axis=1), dtype=np.float16
            )
        )
    FO = [f + 2 * H for f in FEAT]
    RW = [math.ceil((f + H) * 2 / 256) * 128 for f in FEAT]

    # ---- edge partitioning --------------------------------------------------
    owner = dst // NOWN
    src_c = src // NOWN
    src_r = src % NOWN
    # gather-table row index of each edge's src, per chunk
    half_all = (src_r >= C0N).astype(np.int64)
    tabidx = np.where(half_all == 0, src_c * C0N + src_r, src_c * C1N + src_r - C0N)
    per_core = []
    cnt = np.zeros((NCORES, G, 2), dtype=np.int64)
    for c in range(NCORES):
        sel = np.nonzero(owner == c)[0]
        e_src = tabidx[sel]
        dloc = dst[sel] - c * NOWN
        half = half_all[sel]
        grp = dloc // P
        order = np.lexsort((e_src, half, grp))
        e_src, dloc, half, grp = e_src[order], dloc[order], half[order], grp[order]
        np.add.at(cnt[c], (grp, half), 1)
        per_core.append((e_src, dloc, half, grp))

    kmax = np.maximum(np.ceil(cnt.max(axis=0) / P).astype(np.int64), 1)  # [G, 2]
    kA, kB = kmax[:, 0], kmax[:, 1]
    TA, TB = int(kA.sum()), int(kB.sum())
    offA = np.concatenate([[0], np.cumsum(kA)])
    offB = np.concatenate([[0], np.cumsum(kB)])
    offK = np.concatenate([[0], np.cumsum(kA + kB)])
    TK = int(offK[-1])

    in_maps = []
    for c in range(NCORES):
        e_src, dloc, half, grp = per_core[c]
        idxA = np.zeros(TA * P, dtype=np.int64)
        idxB = np.zeros(TB * P, dtype=np.int64)
        idxEA = np.zeros(TA * P, dtype=np.int64)
        idxEB = np.zeros(TB * P, dtype=np.int64)
        dstf = np.full((TK, P), -1.0, dtype=np.float16)
        pos = 0
        for g in range(G):
            for s, (idxS, idxES, offS, kS) in enumerate(
                ((idxA, idxEA, offA, kA), (idxB, idxEB, offB, kB))
            ):
                n = int(cnt[c, g, s])
                ss = e_src[pos : pos + n]
                dd = dloc[pos : pos + n]
                pos += n
                idxS[offS[g] * P : offS[g] * P + n] = ss
                idxES[offS[g] * P : offS[g] * P + n] = dd
                t0 = offK[g] + (kA[g] if s else 0)
                d2 = dstf[t0 : t0 + kS[g]].reshape(-1)
                d2[:n] = (dd - g * P).astype(np.float16)

        kmaxK = int((kA + kB).max())
        iota_big = np.broadcast_to(
            np.arange(P, dtype=np.float16)[None, None, :], (P, kmaxK, P)
        ).reshape(P, kmaxK * P)

        m = {
            "hT": np.ascontiguousarray(
                h[c * NOWN : (c + 1) * NOWN, :].T, dtype=np.float16
            ),
            "iotab": np.ascontiguousarray(iota_big),
            "ident": np.eye(P, dtype=np.float16),
            "dstf": np.ascontiguousarray(dstf.T),  # [128, TK] fp16
            "idxA": np.ascontiguousarray(_wrap_idxs(idxA)),
            "idxB": np.ascontiguousarray(_wrap_idxs(idxB)),
            "idxEA": np.ascontiguousarray(_wrap_idxs(idxEA)),
            "idxEB": np.ascontiguousarray(_wrap_idxs(idxEB)),
            "Wa1": Waug[0],
            "Wa2": Waug[1],
            "Wa3": Waug[2],
        }
        in_maps.append(m)

    cfg = dict(
        N=N,
        E=E,
        NFEAT=NFEAT,
        NOWN=NOWN,
        G=G,
        C0G=C0G,
        H=H,
        FEAT=tuple(FEAT),
        D=tuple(D),
        FO=tuple(FO),
        RW=tuple(RW),
        NCLASS=NCLASS,
        kA=tuple(int(x) for x in kA),
        kB=tuple(int(x) for x in kB),
    )
    return cfg, in_maps


# ----------------------------------------------------------------------------
# Bass program
# ----------------------------------------------------------------------------


def _build(cfg):
    import concourse.bacc as bacc
    import concourse.mybir as mybir
    import concourse.tile as tile

    NOWN, G, C0G = cfg["NOWN"], cfg["G"], cfg["C0G"]
    C0N = min(C0G * P, NOWN)
    C1N = NOWN - C0N
    N, NFEAT, H = cfg["N"], cfg["NFEAT"], cfg["H"]
    FEAT, FO, RW, D = cfg["FEAT"], cfg["FO"], cfg["RW"], cfg["D"]
    NCLASS = cfg["NCLASS"]
    kA, kB = cfg["kA"], cfg["kB"]
    offA = [0]
    offB = [0]
    for g in range(G):
        offA.append(offA[-1] + kA[g])
        offB.append(offB[-1] + kB[g])
    TA, TB = offA[-1], offB[-1]
    TK = TA + TB
    kmaxK = max(kA[g] + kB[g] for g in range(G))
    NEG = 0.2
    f32 = mybir.dt.float32
    f16 = mybir.dt.float16
    f8 = mybir.dt.float8e4  # one-hot dtype: 0/1 exact, half the DVE bytes
    i16 = mybir.dt.int16
    AF = mybir.ActivationFunctionType
    OP = mybir.AluOpType

    F_IN = [NFEAT, FEAT[0], FEAT[1]]
    KT = [math.ceil(f / P) for f in F_IN]
    KTmax = max(KT)
    RWmax = max(RW)

    nc = bacc.Bacc(
        "TRN2",
        target_bir_lowering=False,
        debug=False,
        num_devices=NCORES,
        num_swdge_queues=NQ,
    )

    # ---- I/O ----------------------------------------------------------------
    hT_d = nc.dram_tensor("hT", [NFEAT, NOWN], f16, kind="ExternalInput")
    iotab_d = nc.dram_tensor("iotab", [P, kmaxK * P], f16, kind="ExternalInput")
    ident_d = nc.dram_tensor("ident", [P, P], f16, kind="ExternalInput")
    dstf_d = nc.dram_tensor("dstf", [P, TK], f16, kind="ExternalInput")
    idxA_d = nc.dram_tensor("idxA", [P, TA * 8], i16, kind="ExternalInput")
    idxB_d = nc.dram_tensor("idxB", [P, TB * 8], i16, kind="ExternalInput")
    idxEA_d = nc.dram_tensor("idxEA", [P, TA * 8], i16, kind="ExternalInput")
    idxEB_d = nc.dram_tensor("idxEB", [P, TB * 8], i16, kind="ExternalInput")
    W_d = [
        nc.dram_tensor(f"Wa{i + 1}", [F_IN[i], FO[i]], f16, kind="ExternalInput")
        for i in range(3)
    ]
    out_d = nc.dram_tensor("out", [NOWN, NCLASS], f32, kind="ExternalOutput")

    ag_in0 = [
        nc.dram_tensor(f"ag_in0_{i}", [C0N, RW[i]], f16, kind="Internal")
        for i in range(3)
    ]
    ag_in1 = [
        nc.dram_tensor(f"ag_in1_{i}", [C1N, RW[i]], f16, kind="Internal")
        for i in range(3)
    ]
    ag_out0 = [
        nc.dram_tensor(
            f"ag_out0_{i}",
            [NCORES * C0N, RW[i]],
            f16,
            kind="Internal",
            addr_space="Shared",
        )
        for i in range(3)
    ]
    ag_out1 = [
        nc.dram_tensor(
            f"ag_out1_{i}",
            [NCORES * C1N, RW[i]],
            f16,
            kind="Internal",
            addr_space="Shared",
        )
        for i in range(3)
    ]
    er_wide = [
        nc.dram_tensor(f"er_wide{i}", [NOWN, ERW], f16, kind="Internal")
        for i in range(3)
    ]

    rg = [list(range(NCORES))]

    with tile.TileContext(nc, num_cores=NCORES) as tc:
        with (
            tc.tile_pool(name="const", bufs=1) as cpool,
            tc.tile_pool(name="work", bufs=3) as wpool,
            tc.tile_pool(name="gath", bufs=FBUFS) as gpool,
            tc.tile_pool(name="psum", bufs=2, space="PSUM") as pspool,
        ):
            iotab_t = cpool.tile([P, kmaxK * P], f16, name="iotab_t")
            ident_t = cpool.tile([P, P], f16, name="ident_t")
            dstf_t = cpool.tile([P, TK], f16, name="dstf_t")
            idxA_t = cpool.tile([P, TA * 8], i16, name="idxA_t")
            idxB_t = cpool.tile([P, TB * 8], i16, name="idxB_t")
            idxEA_t = cpool.tile([P, TA * 8], i16, name="idxEA_t")
            idxEB_t = cpool.tile([P, TB * 8], i16, name="idxEB_t")
            nc.sync.dma_start(iotab_t[:], iotab_d[:])
            nc.sync.dma_start(ident_t[:], ident_d[:])
            nc.sync.dma_start(dstf_t[:], dstf_d[:])
            nc.sync.dma_start(idxA_t[:], idxA_d[:])
            nc.sync.dma_start(idxB_t[:], idxB_d[:])
            nc.sync.dma_start(idxEA_t[:], idxEA_d[:])
            nc.sync.dma_start(idxEB_t[:], idxEB_d[:])

            # tiny constants: eps (with 1/H fold for the last layer), ones
            epsc = cpool.tile([P, 2], f32, name="epsc")
            nc.vector.memset(epsc[:, 0:1], 1e-30)
            nc.vector.memset(epsc[:, 1:2], 0.0)
            onec = cpool.tile([P, 1], f16, name="onec")
            nc.vector.memset(onec[:], 1.0)

            W_t = []
            for l in range(3):
                slices = []
                for k in range(KT[l]):
                    r0 = k * P
                    r1 = min(r0 + P, F_IN[l])
                    w = cpool.tile([P, FO[l]], f16, name=f"W{l}_{k}")
                    nc.sync.dma_start(w[: r1 - r0, :], W_d[l][r0:r1, :])
                    slices.append(w)
                W_t.append(slices)

            # x^T split per table chunk so chunk-0 phase A of layer l+1 can
            # start (and its AllGather launch) while layer l's edge phase is
            # still processing chunk-1 groups.
            xT0 = [cpool.tile([P, C0N], f16, name=f"xT0_{k}") for k in range(KTmax)]
            xT1 = [cpool.tile([P, C1N], f16, name=f"xT1_{k}") for k in range(KTmax)]
            for k in range(KT[0]):
                r0, r1 = k * P, min((k + 1) * P, NFEAT)
                nc.sync.dma_start(xT0[k][: r1 - r0, :], hT_d[r0:r1, 0:C0N])
                nc.sync.dma_start(xT1[k][: r1 - r0, :], hT_d[r0:r1, C0N:NOWN])

            def phase_a(ll, chunk):
                FTl, FOl, RWl = FEAT[ll], FO[ll], RW[ll]
                FHl = FTl + H
                g_lo = 0 if chunk == 0 else C0G
                g_hi = C0G if chunk == 0 else G
                ag_in_c = (ag_in0 if chunk == 0 else ag_in1)[ll]
                xTc = xT0 if chunk == 0 else xT1
                base = 0 if chunk == 0 else C0N
                for g in range(g_lo, g_hi):
                    nn = min(P, NOWN - g * P)
                    r0 = g * P - base
                    psA = pspool.tile([P, FOl], f32, name="psA", tag="psA")
                    for k in range(KT[ll]):
                        kk = min(P, F_IN[ll] - k * P)
                        nc.tensor.matmul(
                            psA[:nn, :],
                            lhsT=xTc[k][:kk, r0 : r0 + nn],
                            rhs=W_t[ll][k][:kk, :],
                            start=(k == 0),
                            stop=(k == KT[ll] - 1),
                        )
                    stage = wpool.tile([P, RWl], f16, name="stage", tag="stage")
                    nc.scalar.copy(stage[:nn, 0:FHl], psA[:nn, 0:FHl])
                    if RWl > FHl:
                        nc.vector.memset(stage[:, FHl:RWl], 0.0)
                    nc.sync.dma_start(ag_in_c[r0 : r0 + nn, :], stage[:nn, :])
                    erw = wpool.tile([P, ERW], f16, name="erw", tag="erw")
                    nc.scalar.copy(erw[:nn, 0:H], psA[:nn, FHl:FOl])
                    nc.vector.memset(erw[:, H:ERW], 0.0)
                    nc.sync.dma_start(
                        er_wide[ll][g * P : g * P + nn, :], erw[:nn, :]
                    )
                if chunk == 0:
                    nc.gpsimd.collective_compute(
                        "AllGather",
                        mybir.AluOpType.bypass,
                        replica_groups=rg,
                        ins=[ag_in_c[:]],
                        outs=[ag_out0[ll][:]],
                    )

            qi = 0  # SWDGE queue round-robin

            for l in range(3):
                FT, FOL, RWL, DL = FEAT[l], FO[l], RW[l], D[l]
                FH = FT + H
                last = l == 2

                # Phase A for layer 0 here; later layers' phase A is emitted
                # inside the PREVIOUS layer's edge phase (per chunk) so the
                # chunk-0 AllGather overlaps the previous edge phase.
                if l == 0:
                    phase_a(0, 0)
                    phase_a(0, 1)

                tabA = ag_out0[l][:]
                tabB = ag_out1[l][:]

                # ---------------- Edge phase --------------------------------
                # windowed gathers: window w of stream S covers tiles
                # [w*WIN, min((w+1)*WIN, T_S)); fb and er emitted separately
                # so stream-A feat gathers can overlap the chunk-1 AllGather.
                fbbuf = {}
                ebbuf = {}
                em_fb = [0, 0]
                em_er = [0, 0]
                nwin = [
                    (TA + WIN - 1) // WIN,
                    (TB + WIN - 1) // WIN,
                ]

                def emit_fb(s):
                    w = em_fb[s]
                    T_S = TA if s == 0 else TB
                    t0 = w * WIN
                    tc_ = min(WIN, T_S - t0)
                    nonlocal qi
                    fbw = gpool.tile(
                        [P, WIN * RWL], f16, name=f"fb{s}", tag=f"fb{s}"
                    )
                    nc.gpsimd.dma_gather(
                        fbw[:].rearrange("p (k r) -> p k r", r=RWL)[:, 0:tc_, :],
                        tabA if s == 0 else tabB,
                        (idxA_t if s == 0 else idxB_t)[
                            :, t0 * 8 : (t0 + tc_) * 8
                        ],
                        tc_ * P,
                        tc_ * P,
                        RWL,
                        elem_step=RWL,
                        single_packet=False,
                        queue_num=qi % NQ,
                    )
                    qi += 1
                    fbbuf[(s, w)] = fbw
                    em_fb[s] = w + 1

                def emit_er(s):
                    w = em_er[s]
                    T_S = TA if s == 0 else TB
                    t0 = w * WIN
                    tc_ = min(WIN, T_S - t0)
                    nonlocal qi
                    ebw = gpool.tile(
                        [P, WIN * ERW], f16, name=f"eb{s}", tag=f"eb{s}"
                    )
                    nc.gpsimd.dma_gather(
                        ebw[:].rearrange("p (k r) -> p k r", r=ERW)[:, 0:tc_, :],
                        er_wide[l][:],
                        (idxEA_t if s == 0 else idxEB_t)[
                            :, t0 * 8 : (t0 + tc_) * 8
                        ],
                        tc_ * P,
                        tc_ * P,
                        ERW,
                        elem_step=ERW,
                        single_packet=False,
                        queue_num=qi % NQ,
                    )
                    qi += 1
                    ebbuf[(s, w)] = ebw
                    em_er[s] = w + 1

                def prefetch_fb(s, upto):
                    while em_fb[s] < min(upto, nwin[s]):
                        emit_fb(s)

                def prefetch_er(s, upto):
                    while em_er[s] < min(upto, nwin[s]):
                        emit_er(s)

                def segments(s, g):
                    """Yield (fbw, ebw, w0, t0, t1) covering stream-s tiles of
                    group g, split at window boundaries. t0/t1 are global tile
                    indices; w0 the in-window start offset."""
                    off = offA if s == 0 else offB
                    t = off[g]
                    while t < off[g + 1]:
                        w = t // WIN
                        prefetch_fb(s, w + FBUFS)  # stay FBUFS windows ahead
                        prefetch_er(s, w + FBUFS)
                        t1 = min(off[g + 1], (w + 1) * WIN)
                        yield fbbuf[(s, w)], ebbuf[(s, w)], t - w * WIN, t, t1
                        t = t1

                # er gathers depend only on phase A (not the collectives) —
                # issue them FIRST so they transfer while the chunk-0
                # AllGather is still in flight (the in-order gpsimd queue
                # would otherwise park them behind fbA's wait on it). Then
                # stream-A feat gathers (need chunk-0 AllGather only) before
                # the chunk-1 collective so they overlap it.
                prefetch_er(0, FBUFS)
                prefetch_er(1, FBUFS)
                prefetch_fb(0, FBUFS)
                nc.gpsimd.collective_compute(
                    "AllGather",
                    mybir.AluOpType.bypass,
                    replica_groups=rg,
                    ins=[ag_in1[l][:]],
                    outs=[ag_out1[l][:]],
                )
                prefetch_fb(1, FBUFS)

                def edge_groups():
                    for g in range(G):
                        if g == C0G and l < 2:
                            # chunk-0 of next layer's phase A + its AllGather,
                            # overlapping this layer's remaining edge groups
                            phase_a(l + 1, 0)
                        yield g
                    if l < 2:
                        phase_a(l + 1, 1)

                for g in edge_groups():
                    nn = min(P, NOWN - g * P)
                    K = kA[g] + kB[g]
                    segs = [list(segments(0, g)), list(segments(1, g))]

                    # batched one-hot build for all K tiles of this group
                    ohb = wpool.tile(
                        [P, kmaxK * P], f8, name="ohb", tag="ohb", bufs=2
                    )
                    nc.vector.tensor_tensor(
                        out=ohb[:, 0 : K * P].rearrange("p (k q) -> p k q", q=P),
                        in0=dstf_t[:, offK_g(offA, offB, g) : offK_g(offA, offB, g) + K]
                        .to_broadcast([P, K, P]),
                        in1=iotab_t[:, 0 : K * P].rearrange("p (k q) -> p k q", q=P),
                        op=OP.is_equal,
                    )

                    # batched logits: ee = exp(lrelu(el + er)) -> fs el cols
                    ee = wpool.tile([P, kmaxK * H], f32, name="ee", tag="ee", bufs=2)
                    fs = wpool.tile(
                        [P, kmaxK * FH], f16, name="fs", tag="fs", bufs=2
                    )
                    kbase = [0, kA[g]]
                    for s in (0, 1):
                        for fbw, ebw, w0, t0, t1 in segs[s]:
                            nt = t1 - t0
                            kk0 = kbase[s] + (t0 - (offA if s == 0 else offB)[g])
                            f3 = fbw[:].rearrange("p (k r) -> p k r", r=RWL)
                            e3 = ebw[:].rearrange("p (k r) -> p k r", r=ERW)
                            nc.vector.tensor_add(
                                ee[:, kk0 * H : (kk0 + nt) * H].rearrange(
                                    "p (k h) -> p k h", h=H
                                ),
                                f3[:, w0 : w0 + nt, FT:FH],
                                e3[:, w0 : w0 + nt, 0:H],
                            )
                    nc.vector.scalar_tensor_tensor(
                        out=ee[:, 0 : K * H],
                        in0=ee[:, 0 : K * H],
                        scalar=NEG,
                        in1=ee[:, 0 : K * H],
                        op0=OP.mult,
                        op1=OP.max,
                    )
                    nc.scalar.activation(
                        fs[:, 0 : K * FH].rearrange("p (k f) -> p k f", f=FH)[
                            :, :, FT:FH
                        ],
                        ee[:, 0 : K * H].rearrange("p (k h) -> p k h", h=H),
                        AF.Exp,
                    )
                    # batched alpha*feat
                    f4 = fs[:].rearrange("p (k f) -> p k f", f=FH)
                    for s in (0, 1):
                        for fbw, ebw, w0, t0, t1 in segs[s]:
                            nt = t1 - t0
                            kk0 = kbase[s] + (t0 - (offA if s == 0 else offB)[g])
                            f3 = fbw[:].rearrange("p (k r) -> p k r", r=RWL)
                            nc.vector.tensor_mul(
                                f4[:, kk0 : kk0 + nt, 0:FT].rearrange(
                                    "p k (h d) -> p k h d", h=H
                                ),
                                f3[:, w0 : w0 + nt, 0:FT].rearrange(
                                    "p k (h d) -> p k h d", h=H
                                ),
                                f4[:, kk0 : kk0 + nt, FT:FH].to_broadcast(
                                    [P, nt, H, DL]
                                ),
                            )

                    ps_out = pspool.tile([P, FH], f32, name="ps_out", tag="ps_out")
                    for t in range(K):
                        nc.tensor.matmul(
                            ps_out[:],
                            lhsT=ohb[:, t * P : (t + 1) * P],
                            rhs=fs[:, t * FH : (t + 1) * FH],
                            start=(t == 0),
                            stop=(t == K - 1),
                        )

                    # s_r = 1/(s*(H if last else 1) + eps)
                    s_r = wpool.tile([P, H], f32, name="s_r", tag="s_r")
                    nc.vector.scalar_tensor_tensor(
                        out=s_r[:],
                        in0=ps_out[:, FT:FH],
                        scalar=float(H) if last else 1.0,
                        in1=epsc[:, 0:1].to_broadcast([P, H]),
                        op0=OP.mult,
                        op1=OP.add,
                    )
                    nc.vector.reciprocal(s_r[:], s_r[:])
                    if last:
                        xg = wpool.tile([P, FT], f32, name="xg", tag="xg")
                        nc.vector.tensor_mul(
                            xg[:].rearrange("p (h d) -> p h d", h=H),
                            ps_out[:, 0:FT].rearrange("p (h d) -> p h d", h=H),
                            s_r[:].to_broadcast([P, H, DL]),
                        )
                        o1 = wpool.tile([P, NCLASS], f32, name="o1", tag="o1")
                        o2 = wpool.tile([P, NCLASS], f32, name="o2", tag="o2")
                        nc.vector.tensor_add(
                            o1[:], xg[:, 0:NCLASS], xg[:, NCLASS : 2 * NCLASS]
                        )
                        nc.vector.tensor_add(
                            o2[:],
                            xg[:, 2 * NCLASS : 3 * NCLASS],
                            xg[:, 3 * NCLASS : 4 * NCLASS],
                        )
                        nc.vector.tensor_add(o1[:], o1[:], o2[:])
                        nc.sync.dma_start(out_d[g * P : g * P + nn, :], o1[:nn, :])
                    else:
                        xg = wpool.tile([P, FT], f16, name="xg16", tag="xg16")
                        nc.vector.tensor_mul(
                            xg[:].rearrange("p (h d) -> p h d", h=H),
                            ps_out[:, 0:FT].rearrange("p (h d) -> p h d", h=H),
                            s_r[:].to_broadcast([P, H, DL]),
                        )
                        # elu(x) = max(x, min(exp(x), 1) - 1)
                        mg = wpool.tile([P, FT], f16, name="mg", tag="mg")
                        nc.scalar.activation(mg[:], xg[:], AF.Exp)
                        nc.vector.tensor_tensor(
                            out=mg[:],
                            in0=mg[:],
                            in1=onec[:].to_broadcast([P, FT]),
                            op=OP.min,
                        )
                        nc.vector.scalar_tensor_tensor(
                            out=xg[:],
                            in0=mg[:],
                            scalar=-1.0,
                            in1=xg[:],
                            op0=OP.add,
                            op1=OP.max,
                        )
                        for kk in range(KT[l + 1]):
                            c0 = kk * P
                            c1 = min(c0 + P, FT)
                            w = c1 - c0
                            pt2 = pspool.tile(
                                [P, P], f16, name="pt2", tag="pt2", bufs=2
                            )
                            nc.tensor.transpose(pt2[:w, :], xg[:, c0:c1], ident_t[:])
                            if g < C0G:
                                nc.vector.tensor_copy(
                                    xT0[kk][:w, g * P : g * P + nn], pt2[:w, :nn]
                                )
                            else:
                                r0x = g * P - C0N
                                nc.vector.tensor_copy(
                                    xT1[kk][:w, r0x : r0x + nn], pt2[:w, :nn]
                                )

    nc.compile()
    return nc


def offK_g(offA, offB, g):
    return offA[g] + offB[g]


# ----------------------------------------------------------------------------
# Driver
# ----------------------------------------------------------------------------

_CACHE = {}


def _get_nc(cfg, mm_f32r=None):
    key = str(sorted(cfg.items()))
    if key not in _CACHE:
        _CACHE[key] = _build(cfg)
    return _CACHE[key]


def _run(inputs, trace=False, mm_f32r=None, use_sim=False, bench_iters=0):
    cfg, in_maps = _prepare(inputs)
    nc = _get_nc(cfg)

    if use_sim:
        from concourse.bass_interp import MultiCoreSim

        sim = MultiCoreSim(nc, num_cores=NCORES, require_finite=False)
        for c in range(NCORES):
            for k, v in in_maps[c].items():
                sim.cores[c].tensor(k)[:] = v
        sim.simulate(check_with_hw=False)
        outs = [np.array(sim.cores[c].tensor("out")) for c in range(NCORES)]
        res = None
    else:
        outs, res = _pjrt_run(nc, in_maps, bench_iters=bench_iters)

    out = np.concatenate(outs, axis=0).astype(np.float32)
    return out, res


def _pjrt_run(nc, in_maps, bench_iters=0):
    """Execute the SPMD program on the 8 axon-tunneled cores via PJRT."""
    import time as _time

    import jax
    import numpy as _np
    from jax.sharding import Mesh, PartitionSpec
    from jax.experimental.shard_map import shard_map

    import concourse.mybir as mybir
    from concourse.bass2jax import (
        _bass_exec_p,
        install_neuronx_cc_hook,
        partition_id_tensor,
    )

    install_neuronx_cc_hook()
    n_cores = len(in_maps)

    partition_name = nc.partition_id_tensor.name if nc.partition_id_tensor else None
    in_names, out_names, out_avals, zero_outs = [], [], [], []
    for alloc in nc.m.functions[0].allocations:
        if not isinstance(alloc, mybir.MemoryLocationSet):
            continue
        name = alloc.memorylocations[0].name
        if alloc.kind == "ExternalInput":
            if name != partition_name:
                in_names.append(name)
        elif alloc.kind == "ExternalOutput":
            shape = tuple(alloc.tensor_shape)
            dtype = mybir.dt.np(alloc.dtype)
            out_names.append(name)
            out_avals.append(jax.core.ShapedArray(shape, dtype))
            zero_outs.append(_np.zeros(shape, dtype))
    n_params = len(in_names)
    n_outs = len(out_avals)
    in_names_all = list(in_names) + list(out_names)
    if partition_name is not None:
        in_names_all.append(partition_name)
    donate = tuple(range(n_params, n_params + n_outs))

    def _body(*args):
        operands = list(args)
        if partition_name is not None:
            operands.append(partition_id_tensor())
        outs = _bass_exec_p.bind(
            *operands,
            out_avals=tuple(out_avals),
            in_names=tuple(in_names_all),
            out_names=tuple(out_names),
            lowering_input_output_aliases=(),
            sim_require_finite=True,
            sim_require_nnan=True,
            nc=nc,
        )
        return tuple(outs)

    devices = jax.devices()[:n_cores]
    mesh = Mesh(_np.asarray(devices), ("core",))
    in_specs = (PartitionSpec("core"),) * (n_params + n_outs)
    out_specs = (PartitionSpec("core"),) * n_outs
    sharded = jax.jit(
        shard_map(
            _body, mesh=mesh, in_specs=in_specs, out_specs=out_specs,
            check_rep=False,
        ),
        donate_argnums=donate,
        keep_unused=True,
    )
    concat_in = [
        _np.concatenate([_np.asarray(in_maps[c][nm]) for c in range(n_cores)], axis=0)
        for nm in in_names
    ]

    def _zeros_dev():
        return [
            jax.device_put(
                _np.zeros((n_cores * z.shape[0], *z.shape[1:]), z.dtype),
                jax.sharding.NamedSharding(mesh, PartitionSpec("core")),
            )
            for z in zero_outs
        ]

    dev_in = [
        jax.device_put(a, jax.sharding.NamedSharding(mesh, PartitionSpec("core")))
        for a in concat_in
    ]

    out_arrs = sharded(*dev_in, *_zeros_dev())
    jax.block_until_ready(out_arrs)

    times = []
    for _ in range(bench_iters):
        zs = _zeros_dev()
        jax.block_until_ready(zs)
        t0 = _time.perf_counter()
        o = sharded(*dev_in, *zs)
        jax.block_until_ready(o)
        times.append(_time.perf_counter() - t0)

    outs = [
        {
            nm: _np.asarray(out_arrs[i]).reshape(n_cores, *out_avals[i].shape)[c]
            for i, nm in enumerate(out_names)
        }
        for c in range(n_cores)
    ]
    res = {"times_s": times, "min_time_ns": int(min(times) * 1e9) if times else None}
    return [o["out"] for o in outs], res


def kernel(**inputs):
    out, _ = _run(inputs, trace=False)
    return out
